# revision 2
# baseline (speedup 1.0000x reference)
"""GAT neighbor-aggregation kernel for Trainium2, 8-core data-parallel.

Math (per batch b):
  vu = ea @ U2 ; iv = ea @ W2
  logits[i,j] = sum_c yita_c * leaky_relu(vu[i,c] + iv[j,c], 0.2)
  alpha = softmax_j(where(adj>0, logits, -1e12))
  out = leaky_relu(alpha @ ea, 0.2)

Kernel decomposition used on device:
  leaky_relu(v) = 0.8*relu(v) + 0.2*v
  logits[i,j] = 0.2*p_i + 0.2*q_j + sum_c (0.8*sign(yita_c)) * relu(s[i,c] + t[j,c])
  with s = vu * |yita| (cols scaled), t = iv * |yita|, p_i dropped (constant
  along softmax rows), q_j folded multiplicatively into the final alpha @ ea
  matmul by pre-scaling ea rows with exp(0.2 q_j); an extra ones*eq column
  yields the softmax denominator through the same matmul.

  All O(e*c) setup quantities (sPair, tT2, eaS, mask) are precomputed on the
  host; the device only does the O(e^2*c) work:
    - pairwise relu(s_i + t_j) tiles in fp16, [c-pair, j] layout (2 i's
      packed into 128 partitions), split between the vector engine (fused
      tensor_scalar add+max, 2x fp16 mode, ~353ns) and the scalar engine
      (Relu with per-partition bias, ~700ns) for load balance,
    - reduction over c on the tensor engine with one-hot-padded +-0.8 sign
      weights, accumulating 16 i-pairs into each 32-row PSUM column group,
      round-robin over groups so consecutive matmuls overlap,
    - the adjacency mask is folded into the same PSUM accumulation as one
      full-width matmul adding -60000 at masked (i,j) (identity lhsT, fp16
      mask rhs), so exp(masked logit) flushes to 0 in fp16,
    - softmax without max-subtraction (logits are bounded, |R| < 8, so fp16
      exp is safe), per-128-col chunks: exp (scalar engine, PSUM->SBUF),
      transpose (PE), copy (vector/scalar), alpha @ eaS matmul (PE),
    - reciprocal of the denominator column + Prelu(1/d * P, 0.2) -> out.

Sharding: core = 2*b + h handles batch b, query rows i in [256h, 256h+256).
"""

import numpy as np
from contextlib import ExitStack

import concourse.bass as bass
import concourse.tile as tile
from concourse import bacc, mybir
from concourse.bass_utils import run_bass_kernel_spmd

F32 = mybir.dt.float32
F16 = mybir.dt.float16
OP = mybir.AluOpType

BSZ, E, C = 4, 512, 64
NCORE = 8
IPC = E // 2          # 256 query rows per core
NPAIR = IPC // 2      # 128 i-pairs per core
NTILE = IPC // 128    # 2 logits tiles of 128 i-rows
ACT_NUM = 20          # pairwise blocks to scalar engine: ACT_NUM out of 64
N_WARM = 24           # PE warmup matmuls issued while input DMAs are in flight
MASKV = -60000.0      # mask add value; exp(-60000) == 0 in fp16/fp32


def _build_program():
    nc = bacc.Bacc(
        "TRN2",
        target_bir_lowering=False,
        debug=False,
        enable_asserts=False,
        num_devices=NCORE,
    )
    tT2_ap = nc.dram_tensor("tT2", [128, E], F16, kind="ExternalInput").ap()
    sPair_ap = nc.dram_tensor("sPair", [128, NPAIR], F32, kind="ExternalInput").ap()
    whot_ap = nc.dram_tensor("whot", [128, 2048], F16, kind="ExternalInput").ap()
    wident_ap = nc.dram_tensor("wident", [128, 128], F16, kind="ExternalInput").ap()
    eaS_ap = nc.dram_tensor("eaS", [128, 4 * (C + 1)], F16, kind="ExternalInput").ap()
    madj_ap = nc.dram_tensor("madj", [128, NTILE * E], F16, kind="ExternalInput").ap()
    out_ap = nc.dram_tensor("out", [IPC, C], F32, kind="ExternalOutput").ap()

    with tile.TileContext(nc) as tc:
        with ExitStack() as ctx:
            singles = ctx.enter_context(tc.tile_pool(name="singles", bufs=1))
            xpool = ctx.enter_context(tc.tile_pool(name="xpool", bufs=10))
            ps_logits = ctx.enter_context(
                tc.tile_pool(name="ps_logits", bufs=2, space="PSUM")
            )
            ps_tp = ctx.enter_context(tc.tile_pool(name="ps_tp", bufs=2, space="PSUM"))
            ps_fm = ctx.enter_context(tc.tile_pool(name="ps_fm", bufs=2, space="PSUM"))
            small = ctx.enter_context(tc.tile_pool(name="small", bufs=4))
            epool = ctx.enter_context(tc.tile_pool(name="epool", bufs=4))
            atpool = ctx.enter_context(tc.tile_pool(name="atpool", bufs=3))

            # ---- PE warmup: no input deps, runs during the DMA fill ----
            warm_sb = singles.tile([128, C], F16, tag="warm")
            nc.vector.memset(warm_sb[:], 0.0)
            warm_ps = ps_fm.tile([C, C], F32, tag="fm")
            for _ in range(N_WARM):
                nc.tensor.matmul(warm_ps[:], lhsT=warm_sb[:, 0:C], rhs=warm_sb[:])

            # ---- input DMAs: spread across idle engine sequencers; the two
            # tensors gating the pairwise loop (tT2, sPair) go first ----
            tT2 = singles.tile([128, E], F16, tag="tT2")
            nc.sync.dma_start(tT2[:], tT2_ap[:])
            sPair = singles.tile([128, NPAIR], F32, tag="sPair")
            nc.sync.dma_start(sPair[:], sPair_ap[:])
            madj_sb = singles.tile([128, NTILE, E], F16, tag="madj")
            nc.sync.dma_start(madj_sb[:], madj_ap.rearrange("p (t j) -> p t j", t=NTILE))
            whot_sb = singles.tile([128, 2048], F16, tag="whot")
            nc.gpsimd.dma_start(whot_sb[:], whot_ap[:])
            ident_sb = singles.tile([128, 128], F16, tag="ident")
            nc.gpsimd.dma_start(ident_sb[:], wident_ap[:])
            eaS = singles.tile([128, 4, C + 1], F16, tag="eaS")
            nc.gpsimd.dma_start(eaS[:], eaS_ap.rearrange("p (ch c) -> p ch c", ch=4))

            # ---- main: per 128-row logits tile ----
            for t in range(NTILE):
                logits_ps = ps_logits.tile([128, E], F32, tag="logits")
                # round-robin over the four 32-row PSUM column groups so
                # consecutive matmuls hit disjoint PE column groups; each
                # (kk, g) uses a distinct lhsT address to force a real
                # LDWEIGHTS into that column group
                act_acc = 0
                for kk in range(16):
                    for g in range(4):
                        p = t * 64 + g * 16 + kk  # global pair index
                        x = xpool.tile([128, E], F16, tag="x")
                        act_acc += ACT_NUM
                        if act_acc >= 64:
                            act_acc -= 64
                            nc.scalar.activation(
                                x[:], tT2[:], mybir.ActivationFunctionType.Relu,
                                bias=sPair[:, p : p + 1], scale=1.0,
                            )
                        else:
                            nc.vector.tensor_scalar(
                                x[:], tT2[:], sPair[:, p : p + 1], 0.0, OP.add, OP.max
                            )
                        v = kk * 4 + g
                        nc.tensor.matmul(
                            logits_ps[32 * g : 32 * g + 32, :],
                            lhsT=whot_sb[:, 32 * v : 32 * v + 32],
                            rhs=x[:],
                            start=(kk == 0),
                            stop=False,
                            tile_position=(0, 32 * g),
                        )
                # adjacency mask: logits += -60000 * (1 - adj), one
                # full-width matmul (identity lhsT) closing the accumulation
                nc.tensor.matmul(
                    logits_ps[:],
                    lhsT=ident_sb[:],
                    rhs=madj_sb[:, t, :],
                    start=False,
                    stop=True,
                    skip_group_check=True,
                )
                # softmax numerator (no max-sub: |logits| < 8 unmasked) and
                # alpha @ eaS, per 128-col chunk so the pipeline overlaps
                fm_ps = ps_fm.tile([128, C + 1], F32, tag="fm")
                for ch in range(4):
                    e_h = epool.tile([128, 128], F16, tag="esb")
                    nc.scalar.activation(
                        e_h[:], logits_ps[:, ch * 128 : (ch + 1) * 128],
                        mybir.ActivationFunctionType.Exp, bias=0.0, scale=1.0,
                    )
                    tp = ps_tp.tile([128, 128], F16, tag="tp")
                    nc.tensor.transpose(tp[:], e_h[:], ident_sb)
                    aT = atpool.tile([128, 128], F16, tag="aT")
                    if ch % 2 == 1:
                        nc.vector.tensor_copy(aT[:], tp[:])
                    else:
                        nc.scalar.copy(aT[:], tp[:])
                    nc.tensor.matmul(
                        fm_ps[:],
                        lhsT=aT[:],
                        rhs=eaS[:, ch, :],
                        start=(ch == 0),
                        stop=(ch == 3),
                    )
                # out = leaky_relu(P / denom) = prelu(P * rec, 0.2), rec > 0
                rec = small.tile([128, 1], F32, tag="rec")
                nc.vector.reciprocal(rec[:], fm_ps[:, C : C + 1])
                out_sb = small.tile([128, C], F32, tag="outsb")
                nc.scalar.activation(
                    out_sb[:], fm_ps[:, 0:C], mybir.ActivationFunctionType.Prelu,
                    bias=0.0, scale=rec[:], alpha=0.2,
                )
                nc.sync.dma_start(out_ap[t * 128 : (t + 1) * 128, :], out_sb[:])

    nc.finalize()
    return nc


_NC = None


def _get_nc():
    global _NC
    if _NC is None:
        _NC = _build_program()
    return _NC


def _host_prep(edge_attr, edge_adj, W_2, U_2, yita):
    edge_attr = np.asarray(edge_attr, dtype=np.float32)
    edge_adj = np.asarray(edge_adj)
    W_2 = np.asarray(W_2, dtype=np.float32)
    U_2 = np.asarray(U_2, dtype=np.float32)
    yita = np.asarray(yita, dtype=np.float32)

    y = yita[:, 0]
    ay = np.abs(y)
    w08 = (0.8 * np.sign(y)).astype(np.float16)
    whot = np.zeros((128, 2048), dtype=np.float16)
    for kk in range(16):
        for g in range(4):
            v = kk * 4 + g
            whot[0:C, 32 * v + 2 * kk] = w08
            whot[C:128, 32 * v + 2 * kk + 1] = w08
    wident = np.eye(128, dtype=np.float16)

    in_maps = []
    for core in range(NCORE):
        b, h = divmod(core, 2)
        ea = edge_attr[b]                      # [E, C]
        vu = ea @ U_2                          # [E, C]
        iv = ea @ W_2                          # [E, C]
        s = vu * ay[None, :]                   # [E, C]
        t = iv * ay[None, :]                   # [E, C]
        q = iv @ y                             # [E]
        eq = np.exp(0.2 * q)                   # [E]

        # sPair[:, p]: rows 0:64 = s[i=2p], rows 64:128 = s[2p+1] (local i)
        sh = s[h * IPC : (h + 1) * IPC]        # [IPC, C]
        sPair = np.empty((128, NPAIR), dtype=np.float32)
        sPair[0:C, :] = sh[0::2].T
        sPair[C:128, :] = sh[1::2].T

        # tT2: [c, j] duplicated into both partition halves
        tT2 = np.empty((128, E), dtype=np.float16)
        tT2[0:C, :] = t.T.astype(np.float16)
        tT2[C:128, :] = tT2[0:C, :]

        # eaS[:, ch, 0:64] = ea * eq (row-scaled), col 64 = eq (denominator)
        eaS = np.empty((128, 4, C + 1), dtype=np.float16)
        for chn in range(4):
            rows = slice(chn * 128, (chn + 1) * 128)
            eaS[:, chn, 0:C] = (ea[rows] * eq[rows, None]).astype(np.float16)
            eaS[:, chn, C] = eq[rows].astype(np.float16)

        # madj[r, t, j] = MASKV where adj == 0 else 0, for i = h*IPC + t*128 + r
        adjh = edge_adj[b, h * IPC : (h + 1) * IPC, :]  # [IPC, E]
        madj = np.where(adjh > 0, np.float16(0.0), np.float16(MASKV))
        madj = madj.reshape(NTILE, 128, E).transpose(1, 0, 2)  # [128, NTILE, E]

        in_maps.append(
            {
                "tT2": tT2,
                "sPair": sPair,
                "whot": whot,
                "wident": wident,
                "eaS": np.ascontiguousarray(eaS.reshape(128, 4 * (C + 1))),
                "madj": np.ascontiguousarray(madj.reshape(128, NTILE * E)),
            }
        )
    return in_maps


def kernel(edge_attr, edge_adj, e_max=None, mask=None, W_2=None, U_2=None, yita=None):
    nc = _get_nc()
    in_maps = _host_prep(edge_attr, edge_adj, W_2, U_2, yita)
    res = run_bass_kernel_spmd(nc, in_maps, core_ids=list(range(NCORE)))
    out = np.empty((BSZ, E, C), dtype=np.float32)
    for core in range(NCORE):
        b, h = divmod(core, 2)
        out[b, h * IPC : (h + 1) * IPC, :] = res.results[core]["out"]
    return out


# revision 6
# speedup vs baseline: 3.6667x; 3.6667x over previous
"""GAT neighbor-aggregation kernel for Trainium2, 8-core data-parallel.

Math (per batch b):
  vu = ea @ U2 ; iv = ea @ W2
  logits[i,j] = sum_c yita_c * leaky_relu(vu[i,c] + iv[j,c], 0.2)
  alpha = softmax_j(where(adj>0, logits, -1e12))
  out = leaky_relu(alpha @ ea, 0.2)

Kernel decomposition used on device:
  leaky_relu(v) = 0.8*relu(v) + 0.2*v
  logits[i,j] = 0.2*p_i + 0.2*q_j + sum_c (0.8*sign(yita_c)) * relu(s[i,c] + t[j,c])
  with s = vu * |yita|, t = iv * |yita|; p_i dropped (softmax row constant);
  exp(0.2 q_j) folded multiplicatively into the final alpha @ ea matmul by
  pre-scaling ea rows.  All O(e*c) setup (sPair, tT2, eaS, mask) is
  precomputed on the host; the device does only the O(e^2*c) work:
    - pairwise relu(s_i + t_j) tiles in fp16, [c-pair, j] layout (2 i's in
      128 partitions), split between the vector engine (fused tensor_scalar
      add+max, 2x fp16, ~350ns/tile) and the scalar engine (Relu with
      per-partition bias, ~620ns/tile) by an explicit per-pair schedule that
      keeps the scalar engine free around its exp bursts,
    - c-reduction on the tensor engine with one-hot +-0.8 sign weights,
      round-robin over the four 32-row PSUM column groups,
    - adjacency mask folded into the same PSUM accumulation as one
      full-width matmul adding -60000 at masked (i,j) (identity lhsT),
    - softmax without max-subtraction (|logits| < 8 so fp16 exp is safe);
      exp emits the row-sum denominator via accum_out; tile 0's alpha
      transposes go through the DMA xbar (idle engines), tile 1's through
      the PE + vector-engine copies for a short tail,
    - out = Prelu(P * 1/denom, 0.2) in fp16.

Sharding: core = 2*b + h handles batch b, query rows i in [256h, 256h+256).
"""

import numpy as np
from contextlib import ExitStack

import concourse.bass as bass
import concourse.tile as tile
from concourse import bacc, mybir
from concourse.bass_utils import run_bass_kernel_spmd

F32 = mybir.dt.float32
F16 = mybir.dt.float16
OP = mybir.AluOpType

BSZ, E, C = 4, 512, 64
NCORE = 8
IPC = E // 2          # 256 query rows per core
NPAIR = IPC // 2      # 128 i-pairs per core
NTILE = IPC // 128    # 2 logits tiles of 128 i-rows
N_WARM = 24           # PE warmup matmuls issued while input DMAs are in flight
MASKV = -60000.0      # mask add value; exp(-60000) == 0 in fp16/fp32


def _pair_schedule():
    """Engine per pair: True = scalar engine (ACT), False = vector (DVE).

    DVE-only windows: pairs 0-5 (ACT table load), 70-77 (tile-0 exp burst),
    120-127 (free ACT for tile-1 exps + fast tail).  ~41 ACT tiles total.
    """
    sched = [False] * NPAIR
    acc = 0
    for p in range(NPAIR):
        if p < 6 or 70 <= p < 78 or p >= 120:
            continue
        acc += 25
        if acc >= 64:
            acc -= 64
            sched[p] = True
    return sched


SCHED = _pair_schedule()


def _build_program():
    nc = bacc.Bacc(
        "TRN2",
        target_bir_lowering=False,
        debug=False,
        enable_asserts=False,
        num_devices=NCORE,
    )
    tT2_ap = nc.dram_tensor("tT2", [128, E], F16, kind="ExternalInput").ap()
    sPair_ap = nc.dram_tensor("sPair", [128, NPAIR], F32, kind="ExternalInput").ap()
    whot_ap = nc.dram_tensor("whot", [128, 2048], F16, kind="ExternalInput").ap()
    wident_ap = nc.dram_tensor("wident", [128, 128], F16, kind="ExternalInput").ap()
    eaS_ap = nc.dram_tensor("eaS", [128, 4 * C], F16, kind="ExternalInput").ap()
    madj_ap = nc.dram_tensor("madj", [128, NTILE * E], F16, kind="ExternalInput").ap()
    out_ap = nc.dram_tensor("out", [IPC, C], F16, kind="ExternalOutput").ap()

    with tile.TileContext(nc) as tc:
        with ExitStack() as ctx:
            singles = ctx.enter_context(tc.tile_pool(name="singles", bufs=1))
            xpool = ctx.enter_context(tc.tile_pool(name="xpool", bufs=12))
            ps_logits = ctx.enter_context(
                tc.tile_pool(name="ps_logits", bufs=2, space="PSUM")
            )
            ps_tp = ctx.enter_context(tc.tile_pool(name="ps_tp", bufs=2, space="PSUM"))
            ps_fm = ctx.enter_context(tc.tile_pool(name="ps_fm", bufs=2, space="PSUM"))
            small = ctx.enter_context(tc.tile_pool(name="small", bufs=6))
            epool = ctx.enter_context(tc.tile_pool(name="epool", bufs=4))
            atpool = ctx.enter_context(tc.tile_pool(name="atpool", bufs=4))

            # ---- PE warmup: no input deps, runs during the DMA fill ----
            warm_sb = singles.tile([128, C], F16, tag="warm")
            nc.vector.memset(warm_sb[:], 0.0)
            warm_ps = ps_fm.tile([C, C], F32, tag="fm")
            for _ in range(N_WARM):
                nc.tensor.matmul(warm_ps[:], lhsT=warm_sb[:, 0:C], rhs=warm_sb[:])

            # ---- input DMAs: issue in parallel across the three DMA-capable
            # queues; the tensors gating the pairwise loop (tT2, sPair) and
            # the big whot go first on their queues ----
            tT2 = singles.tile([128, E], F16, tag="tT2")
            nc.sync.dma_start(tT2[:], tT2_ap[:])
            sPair = singles.tile([128, NPAIR], F32, tag="sPair")
            nc.scalar.dma_start(sPair[:], sPair_ap[:])
            whot_sb = singles.tile([128, 2048], F16, tag="whot")
            nc.gpsimd.dma_start(whot_sb[:], whot_ap[:])
            madj_sb = singles.tile([128, NTILE, E], F16, tag="madj")
            nc.sync.dma_start(madj_sb[:], madj_ap.rearrange("p (t j) -> p t j", t=NTILE))
            ident_sb = singles.tile([128, 128], F16, tag="ident")
            nc.gpsimd.dma_start(ident_sb[:], wident_ap[:])
            eaS = singles.tile([128, 4, C], F16, tag="eaS")
            nc.gpsimd.dma_start(eaS[:], eaS_ap.rearrange("p (ch c) -> p ch c", ch=4))

            # ---- main: per 128-row logits tile ----
            for t in range(NTILE):
                logits_ps = ps_logits.tile([128, E], F32, tag="logits")
                # round-robin over the four 32-row PSUM column groups so
                # consecutive matmuls hit disjoint PE column groups; each
                # (kk, g) uses a distinct lhsT address to force a real
                # LDWEIGHTS into that column group
                for kk in range(16):
                    for g in range(4):
                        p = t * 64 + g * 16 + kk  # global pair index
                        x = xpool.tile([128, E], F16, tag="x")
                        if SCHED[p]:
                            nc.scalar.activation(
                                x[:], tT2[:], mybir.ActivationFunctionType.Relu,
                                bias=sPair[:, p : p + 1], scale=1.0,
                            )
                        else:
                            nc.vector.tensor_scalar(
                                x[:], tT2[:], sPair[:, p : p + 1], 0.0, OP.add, OP.max
                            )
                        v = kk * 4 + g
                        nc.tensor.matmul(
                            logits_ps[32 * g : 32 * g + 32, :],
                            lhsT=whot_sb[:, 32 * v : 32 * v + 32],
                            rhs=x[:],
                            start=(kk == 0),
                            stop=False,
                            tile_position=(0, 32 * g),
                        )
                # mask + column bias: logits += -60000 * (1 - adj) + 0.2*q_j,
                # one full-width matmul (identity lhsT) closing the group
                nc.tensor.matmul(
                    logits_ps[:],
                    lhsT=ident_sb[:],
                    rhs=madj_sb[:, t, :],
                    start=False,
                    stop=True,
                    skip_group_check=True,
                )
                # softmax numerator (no max-sub) per 128-col chunk; each exp
                # also emits its partial row-sum (denominator) via accum_out
                fm_ps = ps_fm.tile([128, C], F32, tag="fm")
                dparts = small.tile([128, 4], F32, tag="dparts")
                for ch in range(4):
                    e_h = epool.tile([128, 128], F16, tag="esb")
                    nc.scalar.activation(
                        e_h[:], logits_ps[:, ch * 128 : (ch + 1) * 128],
                        mybir.ActivationFunctionType.Exp, bias=0.0, scale=1.0,
                        accum_out=dparts[:, ch : ch + 1],
                    )
                    aT = atpool.tile([128, 128], F16, tag="aT")
                    if t == 0 and not int(__import__("os").environ.get("NODMAT", "0")):
                        # mid-kernel: transpose via the DMA xbar (idle)
                        nc.sync.dma_start_transpose(aT[:], e_h[:])
                    else:
                        # tail: PE transpose + vector-engine copy (fast path)
                        tp = ps_tp.tile([128, 128], F16, tag="tp")
                        nc.tensor.transpose(tp[:], e_h[:], ident_sb)
                        nc.vector.tensor_copy(aT[:], tp[:])
                    nc.tensor.matmul(
                        fm_ps[:],
                        lhsT=aT[:],
                        rhs=eaS[:, ch, :],
                        start=(ch == 0),
                        stop=(ch == 3),
                    )
                # denominator = sum of the 4 partials; reciprocal off the
                # fm critical path
                dsum = small.tile([128, 1], F32, tag="dsum")
                nc.vector.tensor_tensor(
                    dsum[:], dparts[:, 0:1], dparts[:, 1:2], OP.add
                )
                nc.vector.tensor_tensor(dsum[:], dsum[:], dparts[:, 2:3], OP.add)
                nc.vector.tensor_tensor(dsum[:], dsum[:], dparts[:, 3:4], OP.add)
                rec = small.tile([128, 1], F32, tag="rec")
                nc.vector.reciprocal(rec[:], dsum[:])
                # out = leaky_relu(P / denom) = prelu(P * rec, 0.2), rec > 0
                out_sb = small.tile([128, C], F16, tag="outsb")
                nc.scalar.activation(
                    out_sb[:], fm_ps[:], mybir.ActivationFunctionType.Prelu,
                    bias=0.0, scale=rec[:], alpha=0.2,
                )
                nc.sync.dma_start(out_ap[t * 128 : (t + 1) * 128, :], out_sb[:])

    nc.finalize()
    return nc


_NC = None


def _get_nc():
    global _NC
    if _NC is None:
        _NC = _build_program()
    return _NC


def _host_prep(edge_attr, edge_adj, W_2, U_2, yita):
    edge_attr = np.asarray(edge_attr, dtype=np.float32)
    edge_adj = np.asarray(edge_adj)
    W_2 = np.asarray(W_2, dtype=np.float32)
    U_2 = np.asarray(U_2, dtype=np.float32)
    yita = np.asarray(yita, dtype=np.float32)

    y = yita[:, 0]
    ay = np.abs(y)
    w08 = (0.8 * np.sign(y)).astype(np.float16)
    whot = np.zeros((128, 2048), dtype=np.float16)
    for kk in range(16):
        for g in range(4):
            v = kk * 4 + g
            whot[0:C, 32 * v + 2 * kk] = w08
            whot[C:128, 32 * v + 2 * kk + 1] = w08
    wident = np.eye(128, dtype=np.float16)

    in_maps = []
    for core in range(NCORE):
        b, h = divmod(core, 2)
        ea = edge_attr[b]                      # [E, C]
        vu = ea @ U_2                          # [E, C]
        iv = ea @ W_2                          # [E, C]
        s = vu * ay[None, :]                   # [E, C]
        t = iv * ay[None, :]                   # [E, C]
        q = iv @ y                             # [E]
        eq = np.exp(0.2 * q)                   # [E]

        # sPair[:, p]: rows 0:64 = s[i=2p], rows 64:128 = s[2p+1] (local i)
        sh = s[h * IPC : (h + 1) * IPC]        # [IPC, C]
        sPair = np.empty((128, NPAIR), dtype=np.float32)
        sPair[0:C, :] = sh[0::2].T
        sPair[C:128, :] = sh[1::2].T

        # tT2: [c, j] duplicated into both partition halves
        tT2 = np.empty((128, E), dtype=np.float16)
        tT2[0:C, :] = t.T.astype(np.float16)
        tT2[C:128, :] = tT2[0:C, :]

        # eaS[:, ch, :] = ea in chunk layout; the 0.2*q_j column bias rides
        # the mask matmul so exp already carries it (numerator AND the
        # accum_out denominator)
        eaS = np.empty((128, 4, C), dtype=np.float16)
        for chn in range(4):
            rows = slice(chn * 128, (chn + 1) * 128)
            eaS[:, chn, :] = ea[rows].astype(np.float16)

        # madj[r, t, j] = (MASKV if adj == 0 else 0) + 0.2*q_j,
        # for i = h*IPC + t*128 + r
        adjh = edge_adj[b, h * IPC : (h + 1) * IPC, :]  # [IPC, E]
        madj = np.where(adjh > 0, 0.0, MASKV) + 0.2 * q[None, :]
        madj = madj.astype(np.float16)
        madj = madj.reshape(NTILE, 128, E).transpose(1, 0, 2)  # [128, NTILE, E]

        in_maps.append(
            {
                "tT2": tT2,
                "sPair": sPair,
                "whot": whot,
                "wident": wident,
                "eaS": np.ascontiguousarray(eaS.reshape(128, 4 * C)),
                "madj": np.ascontiguousarray(madj.reshape(128, NTILE * E)),
            }
        )
    return in_maps


def kernel(edge_attr, edge_adj, e_max=None, mask=None, W_2=None, U_2=None, yita=None):
    nc = _get_nc()
    in_maps = _host_prep(edge_attr, edge_adj, W_2, U_2, yita)
    res = run_bass_kernel_spmd(nc, in_maps, core_ids=list(range(NCORE)))
    out = np.empty((BSZ, E, C), dtype=np.float32)
    for core in range(NCORE):
        b, h = divmod(core, 2)
        out[b, h * IPC : (h + 1) * IPC, :] = res.results[core]["out"].astype(
            np.float32
        )
    return out
